# revision 2
# baseline (speedup 1.0000x reference)
"""Trainium2 Bass kernel for masked-dropout attention-score matmul.

Computes, for q/k/v [B,H,S,D] and an int32 0/1 keep-mask [B,H,S,S]:

    out = ((q @ k^T) * sqrt(D) * 2 * mask) @ v        (2 = 1/(1-p_drop))

Strategy (8 NeuronCores, SPMD, no collectives):
  - Shard the 32 (b,h) pairs 4-per-core.
  - Per pair, compute S^T = K @ Q^T on the PE (so the second matmul can
    consume it as its moving operand without any on-chip transpose),
    apply the mask during the PSUM->SBUF eviction, and accumulate
    O^T = V^T @ S'^T on the PE.
  - The scale (2*sqrt(D)) is folded into V on the host; mask values are
    shipped as fp8(0/1) bytes; Q^T/K^T/V are host-rearranged so all
    device DMAs are fully contiguous.

v3 (default): bf16 matmul operands (PE ~10% faster than f32r, FWL on),
  [128,1024] two-bank PSUM score groups, and the mask-apply work is
  split across three engines to get the whole masking stage under the
  PE roofline:
    path A: DVE multiplies PSUM f32 scores by the fp8 mask -> bf16 SBUF
    path B: ACT evicts PSUM->bf16, Pool converts mask fp8->bf16, DVE
            multiplies bf16 x bf16 in its 2x mode
    path C: like B but the multiply runs on Pool (gpsimd)
  Output is shipped bf16 and upcast on the host.
"""

import os
import sys

sys.path.insert(0, "/opt/trn_rl_repo")

import numpy as np

import concourse.bacc as bacc
import concourse.bass as bass
import concourse.mybir as mybir
import concourse.tile as tile
from concourse.bass_utils import run_bass_kernel_spmd

B, H, SQ, SK, D = 2, 16, 2048, 2048, 128
P_DROP = 0.5
SCALE = float(D) ** 0.5 / (1.0 - P_DROP)  # folded into V on the host
N_CORES = 8
PAIRS = B * H
PAIRS_PER_CORE = PAIRS // N_CORES

F32 = mybir.dt.float32
F32R = mybir.dt.float32r
FP8 = mybir.dt.float8e4
U8 = mybir.dt.uint8
BF16 = mybir.dt.bfloat16

FP8_ONE = 0x38  # float8_e4m3 encoding of 1.0

# module-level handle for test.py to inspect timing after a traced run
LAST_RESULTS = None


def emit_body(nc, tc, ot, qt, kt, v, mt, n_pairs, sq, sk, d=D, qn=512, repeat=1,
              loop_n=1, mmdt=F32R):
    """v1 per-core program (kept for A/B reference).

    APs (all on this core's DRAM):
      qt [n_pairs, d,  sq]  f32  : Q^T per pair
      kt [n_pairs, d,  sk]  f32  : K^T per pair
      v  [n_pairs, 128, (sk//128)*d] : V rearranged, scaled
      mt [n_pairs, sk, sq]  u8   : mask^T as fp8 bytes (0x00 / 0x38)
      ot [n_pairs, d,  sq]  f32  : O^T output
    """
    nkc = sk // 128
    nqc = sq // qn

    import contextlib

    with contextlib.ExitStack() as ctx:
        qt_pool = ctx.enter_context(tc.tile_pool(name="qt", bufs=2))
        kt_pool = ctx.enter_context(tc.tile_pool(name="kt", bufs=2))
        v_pool = ctx.enter_context(tc.tile_pool(name="v", bufs=2))
        m_pool = ctx.enter_context(tc.tile_pool(name="m", bufs=4))
        sp_pool = ctx.enter_context(tc.tile_pool(name="sp", bufs=6))
        o_pool = ctx.enter_context(tc.tile_pool(name="o", bufs=2))
        st_pool = ctx.enter_context(tc.tile_pool(name="st", bufs=4, space="PSUM"))
        ot_pool = ctx.enter_context(tc.tile_pool(name="otp", bufs=1, space="PSUM"))

        loop_cm = tc.For_i(0, loop_n, 1) if loop_n > 1 else contextlib.nullcontext()
        with loop_cm:
          for p in [pp for _ in range(repeat) for pp in range(n_pairs)]:
            qt_t = qt_pool.tile([128, sq], mmdt)
            nc.sync.dma_start(out=qt_t[:d], in_=qt[p])
            kt_t = kt_pool.tile([128, sk], mmdt)
            nc.sync.dma_start(out=kt_t[:d], in_=kt[p])
            v_t = v_pool.tile([128, nkc * d], mmdt)
            nc.sync.dma_start(out=v_t[:], in_=v[p])

            ot_ps = ot_pool.tile([128, sq], F32)

            for kc in range(nkc):
                m_t = m_pool.tile([128, sq], U8)
                nc.sync.dma_start(out=m_t[:], in_=mt[p, kc * 128 : (kc + 1) * 128, :])

                for qc in range(nqc):
                    st = st_pool.tile([128, qn], F32)
                    nc.tensor.matmul(
                        st[:],
                        kt_t[:d, kc * 128 : (kc + 1) * 128],
                        qt_t[:d, qc * qn : (qc + 1) * qn],
                        start=True,
                        stop=True,
                    )
                    sp = sp_pool.tile([128, qn], mmdt)
                    nc.vector.tensor_mul(
                        sp[:],
                        st[:],
                        m_t[:, qc * qn : (qc + 1) * qn].bitcast(FP8),
                    )
                    nc.tensor.matmul(
                        ot_ps[:d, qc * qn : (qc + 1) * qn],
                        v_t[:, kc * d : (kc + 1) * d],
                        sp[:],
                        start=(kc == 0),
                        stop=(kc == nkc - 1),
                    )

            o_t = o_pool.tile([128, sq], F32)
            nc.scalar.copy(o_t[:d], ot_ps[:d])
            nc.sync.dma_start(out=ot[p], in_=o_t[:d])


def emit_body_v3(nc, tc, ot, qt, kt, v, mt, n_pairs, sq, sk, d=D, gn=1024,
                 loop_n=1, paths=("A", "A", "A", "B", "A", "B", "C", "A",
                                  "B", "A", "B", "C", "A", "B", "A", "C")):
    """Three-engine masking split, bf16 matmuls, bf16 output.

    Per [128, gn=1024] score group (two PSUM banks, two N=512 matmuls each
    side), the PSUM->SBUF eviction + mask-multiply goes down one of:
      A: DVE tensor_mul(PSUM f32, fp8 mask) -> bf16      (~1.24us DVE)
      B: ACT copy -> bf16; Pool fp8->bf16 mask; DVE 2x mul (~1.0/0.73/0.64us)
      C: like B, multiply on Pool                         (~1.0/2.9us ACT/Pool)
    """
    nkc = sk // 128
    ngc = sq // gn
    qn = 512
    import contextlib

    with contextlib.ExitStack() as ctx:
        qt_pool = ctx.enter_context(tc.tile_pool(name="qt", bufs=2))
        kt_pool = ctx.enter_context(tc.tile_pool(name="kt", bufs=2))
        v_pool = ctx.enter_context(tc.tile_pool(name="v", bufs=2))
        m_pool = ctx.enter_context(tc.tile_pool(name="m", bufs=4))
        mb_pool = ctx.enter_context(tc.tile_pool(name="mb", bufs=6))
        se_pool = ctx.enter_context(tc.tile_pool(name="se", bufs=6))
        sp_pool = ctx.enter_context(tc.tile_pool(name="sp", bufs=8))
        o_pool = ctx.enter_context(tc.tile_pool(name="o", bufs=2))
        st_pool = ctx.enter_context(tc.tile_pool(name="st", bufs=2, space="PSUM"))
        ot_pool = ctx.enter_context(tc.tile_pool(name="otp", bufs=1, space="PSUM"))

        unit = 0
        loop_cm = tc.For_i(0, loop_n, 1) if loop_n > 1 else contextlib.nullcontext()
        with loop_cm:
          for p in range(n_pairs):
            qt_t = qt_pool.tile([128, sq], BF16)
            nc.sync.dma_start(out=qt_t[:d], in_=qt[p])
            kt_t = kt_pool.tile([128, sk], BF16)
            nc.sync.dma_start(out=kt_t[:d], in_=kt[p])
            v_t = v_pool.tile([128, nkc * d], BF16)
            nc.sync.dma_start(out=v_t[:], in_=v[p])

            ot_ps = ot_pool.tile([128, sq], F32)

            for kc in range(nkc):
                m_t = m_pool.tile([128, sq], U8)
                nc.sync.dma_start(out=m_t[:], in_=mt[p, kc * 128 : (kc + 1) * 128, :])

                for g in range(ngc):
                    st = st_pool.tile([128, gn], F32)
                    for j in range(gn // qn):
                        c0 = g * gn + j * qn
                        nc.tensor.matmul(
                            st[:, j * qn : (j + 1) * qn],
                            kt_t[:d, kc * 128 : (kc + 1) * 128],
                            qt_t[:d, c0 : c0 + qn],
                            start=True,
                            stop=True,
                        )
                    m_sl = m_t[:, g * gn : (g + 1) * gn].bitcast(FP8)
                    sp = sp_pool.tile([128, gn], BF16)
                    path = paths[unit % len(paths)]
                    unit += 1
                    if path == "A":
                        nc.vector.tensor_mul(sp[:], st[:], m_sl)
                    else:
                        se = se_pool.tile([128, gn], BF16)
                        nc.scalar.copy(se[:], st[:])
                        mb = mb_pool.tile([128, gn], BF16)
                        nc.gpsimd.tensor_copy(mb[:], m_sl)
                        if path == "B":
                            nc.vector.tensor_mul(sp[:], se[:], mb[:])
                        else:
                            nc.gpsimd.tensor_mul(sp[:], se[:], mb[:])
                    for j in range(gn // qn):
                        c0 = g * gn + j * qn
                        nc.tensor.matmul(
                            ot_ps[:d, c0 : c0 + qn],
                            v_t[:, kc * d : (kc + 1) * d],
                            sp[:, j * qn : (j + 1) * qn],
                            start=(kc == 0),
                            stop=(kc == nkc - 1),
                        )

            o_t = o_pool.tile([128, sq], BF16)
            nc.scalar.copy(o_t[:d], ot_ps[:d])
            nc.sync.dma_start(out=ot[p], in_=o_t[:d])


def build_nc(n_pairs=PAIRS_PER_CORE, sq=SQ, sk=SK, d=D, qn=512, variant="v3",
             repeat=1, loop_n=1):
    nc = bacc.Bacc("TRN2", target_bir_lowering=False, debug=False)
    if variant == "v3":
        qt = nc.declare_dram_parameter("qt", [n_pairs, d, sq], BF16, isOutput=False)
        kt = nc.declare_dram_parameter("kt", [n_pairs, d, sk], BF16, isOutput=False)
        v = nc.declare_dram_parameter("v", [n_pairs, 128, (sk // 128) * d], BF16,
                                      isOutput=False)
        mt = nc.declare_dram_parameter("mt", [n_pairs, sk, sq], U8, isOutput=False)
        ot = nc.declare_dram_parameter("ot", [n_pairs, d, sq], BF16, isOutput=True)
        with tile.TileContext(nc) as tc:
            emit_body_v3(nc, tc, ot, qt, kt, v, mt, n_pairs, sq, sk, d,
                         loop_n=loop_n)
    else:
        mmdt = F32R
        qt = nc.declare_dram_parameter("qt", [n_pairs, d, sq], mmdt, isOutput=False)
        kt = nc.declare_dram_parameter("kt", [n_pairs, d, sk], mmdt, isOutput=False)
        v = nc.declare_dram_parameter("v", [n_pairs, 128, (sk // 128) * d], mmdt,
                                      isOutput=False)
        mt = nc.declare_dram_parameter("mt", [n_pairs, sk, sq], U8, isOutput=False)
        ot = nc.declare_dram_parameter("ot", [n_pairs, d, sq], F32, isOutput=True)
        with tile.TileContext(nc) as tc:
            emit_body(nc, tc, ot, qt, kt, v, mt, n_pairs, sq, sk, d, qn,
                      repeat=repeat, loop_n=loop_n, mmdt=mmdt)
    nc.compile()
    return nc


def _prep_inputs(query, key, value, dropout_mask, variant="v3"):
    """Host-side marshaling into per-core input maps."""
    import ml_dtypes

    q = np.asarray(query, dtype=np.float32).reshape(PAIRS, SQ, D)
    k = np.asarray(key, dtype=np.float32).reshape(PAIRS, SK, D)
    vv = np.asarray(value, dtype=np.float32).reshape(PAIRS, SK, D)
    m = np.asarray(dropout_mask).reshape(PAIRS, SQ, SK)

    qt = np.ascontiguousarray(q.transpose(0, 2, 1))  # [PAIRS, D, SQ]
    kt = np.ascontiguousarray(k.transpose(0, 2, 1))  # [PAIRS, D, SK]
    # V * SCALE rearranged: vr[p][r][c*D+j] = V[c*128+r, j] * SCALE
    vr = (vv * np.float32(SCALE)).reshape(PAIRS, SK // 128, 128, D)
    vr = np.ascontiguousarray(vr.transpose(0, 2, 1, 3)).reshape(PAIRS, 128, (SK // 128) * D)
    if variant == "v3":
        qt = qt.astype(ml_dtypes.bfloat16)
        kt = kt.astype(ml_dtypes.bfloat16)
        vr = vr.astype(ml_dtypes.bfloat16)
    # mask^T as fp8 bytes
    mb = (m != 0).astype(np.uint8) * np.uint8(FP8_ONE)  # [PAIRS, SQ, SK] u8
    mbt = np.ascontiguousarray(mb.transpose(0, 2, 1))  # [PAIRS, SK, SQ]

    in_maps = []
    for c in range(N_CORES):
        s = slice(c * PAIRS_PER_CORE, (c + 1) * PAIRS_PER_CORE)
        in_maps.append(
            {
                "qt": qt[s],
                "kt": kt[s],
                "v": vr[s],
                "mt": mbt[s],
            }
        )
    return in_maps


def kernel(query, key, value, dropout_mask):
    global LAST_RESULTS
    variant = os.environ.get("KERNEL_VARIANT", "v3")
    in_maps = _prep_inputs(query, key, value, dropout_mask, variant)
    nc = build_nc(variant=variant)
    res = run_bass_kernel_spmd(nc, in_maps, list(range(N_CORES)), trace=False)
    LAST_RESULTS = res
    outs = np.concatenate([r["ot"] for r in res.results], axis=0)  # [PAIRS, D, SQ]
    out = outs.astype(np.float32).transpose(0, 2, 1).reshape(B, H, SQ, D)
    return np.ascontiguousarray(out)


# revision 3
# speedup vs baseline: 1.7400x; 1.7400x over previous
"""Trainium2 Bass kernel for masked-dropout attention-score matmul.

Computes, for q/k/v [B,H,S,D] and an int32 0/1 keep-mask [B,H,S,S]:

    out = ((q @ k^T) * sqrt(D) * 2 * mask) @ v        (2 = 1/(1-p_drop))

Strategy (8 NeuronCores, SPMD, no collectives):
  - Shard the 32 (b,h) pairs 4-per-core.
  - Per pair, compute S^T = K @ Q^T on the PE (so the second matmul can
    consume it as its moving operand without any on-chip transpose),
    apply the mask during the PSUM->SBUF eviction, and accumulate
    O^T = V^T @ S'^T on the PE.
  - The scale (2*sqrt(D)) is folded into V on the host; mask values are
    shipped as fp8(0/1) bytes; Q^T/K^T/V are host-rearranged so all
    device DMAs are fully contiguous.

v3 (default): bf16 matmul operands (PE ~10% faster than f32r, FWL on),
  [128,1024] two-bank PSUM score groups, and the mask-apply work is
  split across three engines to get the whole masking stage under the
  PE roofline:
    path A: DVE multiplies PSUM f32 scores by the fp8 mask -> bf16 SBUF
    path B: ACT evicts PSUM->bf16, Pool converts mask fp8->bf16, DVE
            multiplies bf16 x bf16 in its 2x mode
    path C: like B but the multiply runs on Pool (gpsimd)
  Output is shipped bf16 and upcast on the host.
"""

import os
import sys

sys.path.insert(0, "/opt/trn_rl_repo")

import numpy as np

import concourse.bacc as bacc
import concourse.bass as bass
import concourse.mybir as mybir
import concourse.tile as tile
from concourse.bass_utils import run_bass_kernel_spmd

B, H, SQ, SK, D = 2, 16, 2048, 2048, 128
P_DROP = 0.5
SCALE = float(D) ** 0.5 / (1.0 - P_DROP)  # folded into V on the host
N_CORES = 8
PAIRS = B * H
PAIRS_PER_CORE = PAIRS // N_CORES

F32 = mybir.dt.float32
F32R = mybir.dt.float32r
FP8 = mybir.dt.float8e4
U8 = mybir.dt.uint8
BF16 = mybir.dt.bfloat16

FP8_ONE = 0x38  # float8_e4m3 encoding of 1.0

# module-level handle for test.py to inspect timing after a traced run
LAST_RESULTS = None


def emit_body(nc, tc, ot, qt, kt, v, mt, n_pairs, sq, sk, d=D, qn=512, repeat=1,
              loop_n=1, mmdt=F32R):
    """v1 per-core program (kept for A/B reference).

    APs (all on this core's DRAM):
      qt [n_pairs, d,  sq]  f32  : Q^T per pair
      kt [n_pairs, d,  sk]  f32  : K^T per pair
      v  [n_pairs, 128, (sk//128)*d] : V rearranged, scaled
      mt [n_pairs, sk, sq]  u8   : mask^T as fp8 bytes (0x00 / 0x38)
      ot [n_pairs, d,  sq]  f32  : O^T output
    """
    nkc = sk // 128
    nqc = sq // qn

    import contextlib

    with contextlib.ExitStack() as ctx:
        qt_pool = ctx.enter_context(tc.tile_pool(name="qt", bufs=2))
        kt_pool = ctx.enter_context(tc.tile_pool(name="kt", bufs=2))
        v_pool = ctx.enter_context(tc.tile_pool(name="v", bufs=2))
        m_pool = ctx.enter_context(tc.tile_pool(name="m", bufs=4))
        sp_pool = ctx.enter_context(tc.tile_pool(name="sp", bufs=6))
        o_pool = ctx.enter_context(tc.tile_pool(name="o", bufs=2))
        st_pool = ctx.enter_context(tc.tile_pool(name="st", bufs=4, space="PSUM"))
        ot_pool = ctx.enter_context(tc.tile_pool(name="otp", bufs=1, space="PSUM"))

        loop_cm = tc.For_i(0, loop_n, 1) if loop_n > 1 else contextlib.nullcontext()
        with loop_cm:
          for p in [pp for _ in range(repeat) for pp in range(n_pairs)]:
            qt_t = qt_pool.tile([128, sq], mmdt)
            nc.sync.dma_start(out=qt_t[:d], in_=qt[p])
            kt_t = kt_pool.tile([128, sk], mmdt)
            nc.sync.dma_start(out=kt_t[:d], in_=kt[p])
            v_t = v_pool.tile([128, nkc * d], mmdt)
            nc.sync.dma_start(out=v_t[:], in_=v[p])

            ot_ps = ot_pool.tile([128, sq], F32)

            for kc in range(nkc):
                m_t = m_pool.tile([128, sq], U8)
                nc.sync.dma_start(out=m_t[:], in_=mt[p, kc * 128 : (kc + 1) * 128, :])

                for qc in range(nqc):
                    st = st_pool.tile([128, qn], F32)
                    nc.tensor.matmul(
                        st[:],
                        kt_t[:d, kc * 128 : (kc + 1) * 128],
                        qt_t[:d, qc * qn : (qc + 1) * qn],
                        start=True,
                        stop=True,
                    )
                    sp = sp_pool.tile([128, qn], mmdt)
                    nc.vector.tensor_mul(
                        sp[:],
                        st[:],
                        m_t[:, qc * qn : (qc + 1) * qn].bitcast(FP8),
                    )
                    nc.tensor.matmul(
                        ot_ps[:d, qc * qn : (qc + 1) * qn],
                        v_t[:, kc * d : (kc + 1) * d],
                        sp[:],
                        start=(kc == 0),
                        stop=(kc == nkc - 1),
                    )

            o_t = o_pool.tile([128, sq], F32)
            nc.scalar.copy(o_t[:d], ot_ps[:d])
            nc.sync.dma_start(out=ot[p], in_=o_t[:d])


KC_PATHS = "ABBABABBABABBABB"  # 7 A / 9 B per 16 k-blocks


def emit_body_v3(nc, tc, ot, qt, kt, v, mt, n_pairs, sq, sk, d=D, gn=1024,
                 loop_n=1, kc_paths=KC_PATHS):
    """Two-path masking split, bf16 matmuls, bf16 output.

    Per k-block (one [128, sq] mask slice, two [128, gn=1024] score groups),
    the PSUM->SBUF eviction + mask-multiply goes down one of:
      A: DVE tensor_mul(PSUM f32, fp8 mask) -> bf16 SBUF    (~1.28us DVE/grp)
      B: gpsimd cast-DMA ships the mask slice as bf16 (HBM bytes stay fp8),
         ACT evicts PSUM f32 -> bf16, DVE multiplies bf16 x bf16 in 2x mode
         (~1.2us ACT + ~0.45us DVE per group)
    The A:B ratio balances DVE against ACT so both stay under the PE time.
    """
    nkc = sk // 128
    ngc = sq // gn
    qn = 512
    import contextlib

    with contextlib.ExitStack() as ctx:
        qt_pool = ctx.enter_context(tc.tile_pool(name="qt", bufs=2))
        kt_pool = ctx.enter_context(tc.tile_pool(name="kt", bufs=2))
        v_pool = ctx.enter_context(tc.tile_pool(name="v", bufs=2))
        m_pool = ctx.enter_context(tc.tile_pool(name="m", bufs=4))
        mb_pool = ctx.enter_context(tc.tile_pool(name="mb", bufs=4))
        se_pool = ctx.enter_context(tc.tile_pool(name="se", bufs=6))
        sp_pool = ctx.enter_context(tc.tile_pool(name="sp", bufs=8))
        o_pool = ctx.enter_context(tc.tile_pool(name="o", bufs=2))
        st_pool = ctx.enter_context(tc.tile_pool(name="st", bufs=2, space="PSUM"))
        ot_pool = ctx.enter_context(tc.tile_pool(name="otp", bufs=1, space="PSUM"))

        loop_cm = tc.For_i(0, loop_n, 1) if loop_n > 1 else contextlib.nullcontext()
        with loop_cm:
          for p in range(n_pairs):
            qt_t = qt_pool.tile([128, sq], BF16)
            nc.sync.dma_start(out=qt_t[:d], in_=qt[p])
            kt_t = kt_pool.tile([128, sk], BF16)
            nc.sync.dma_start(out=kt_t[:d], in_=kt[p])
            v_t = v_pool.tile([128, nkc * d], BF16)
            nc.sync.dma_start(out=v_t[:], in_=v[p])

            ot_ps = ot_pool.tile([128, sq], F32)

            for kc in range(nkc):
                path = kc_paths[kc % len(kc_paths)]
                m_src = mt[p, kc * 128 : (kc + 1) * 128, :].bitcast(FP8)
                if path == "A":
                    m_t = m_pool.tile([128, sq], U8)
                    nc.sync.dma_start(out=m_t[:], in_=m_src.bitcast(U8))
                else:
                    mb_t = mb_pool.tile([128, sq], BF16)
                    nc.gpsimd.dma_start(out=mb_t[:], in_=m_src)

                for g in range(ngc):
                    st = st_pool.tile([128, gn], F32)
                    for j in range(gn // qn):
                        c0 = g * gn + j * qn
                        nc.tensor.matmul(
                            st[:, j * qn : (j + 1) * qn],
                            kt_t[:d, kc * 128 : (kc + 1) * 128],
                            qt_t[:d, c0 : c0 + qn],
                            start=True,
                            stop=True,
                        )
                    sp = sp_pool.tile([128, gn], BF16)
                    if path == "A":
                        nc.vector.tensor_mul(
                            sp[:], st[:], m_t[:, g * gn : (g + 1) * gn].bitcast(FP8)
                        )
                    else:
                        se = se_pool.tile([128, gn], BF16)
                        nc.scalar.copy(se[:], st[:])
                        nc.vector.tensor_mul(
                            sp[:], se[:], mb_t[:, g * gn : (g + 1) * gn]
                        )
                    for j in range(gn // qn):
                        c0 = g * gn + j * qn
                        nc.tensor.matmul(
                            ot_ps[:d, c0 : c0 + qn],
                            v_t[:, kc * d : (kc + 1) * d],
                            sp[:, j * qn : (j + 1) * qn],
                            start=(kc == 0),
                            stop=(kc == nkc - 1),
                        )

            o_t = o_pool.tile([128, sq], BF16)
            nc.scalar.copy(o_t[:d], ot_ps[:d])
            nc.sync.dma_start(out=ot[p], in_=o_t[:d])


def build_nc(n_pairs=PAIRS_PER_CORE, sq=SQ, sk=SK, d=D, qn=512, variant="v3",
             repeat=1, loop_n=1):
    nc = bacc.Bacc("TRN2", target_bir_lowering=False, debug=False)
    if variant == "v3":
        qt = nc.declare_dram_parameter("qt", [n_pairs, d, sq], BF16, isOutput=False)
        kt = nc.declare_dram_parameter("kt", [n_pairs, d, sk], BF16, isOutput=False)
        v = nc.declare_dram_parameter("v", [n_pairs, 128, (sk // 128) * d], BF16,
                                      isOutput=False)
        mt = nc.declare_dram_parameter("mt", [n_pairs, sk, sq], U8, isOutput=False)
        ot = nc.declare_dram_parameter("ot", [n_pairs, d, sq], BF16, isOutput=True)
        with tile.TileContext(nc) as tc:
            emit_body_v3(nc, tc, ot, qt, kt, v, mt, n_pairs, sq, sk, d,
                         loop_n=loop_n)
    else:
        mmdt = F32R
        qt = nc.declare_dram_parameter("qt", [n_pairs, d, sq], mmdt, isOutput=False)
        kt = nc.declare_dram_parameter("kt", [n_pairs, d, sk], mmdt, isOutput=False)
        v = nc.declare_dram_parameter("v", [n_pairs, 128, (sk // 128) * d], mmdt,
                                      isOutput=False)
        mt = nc.declare_dram_parameter("mt", [n_pairs, sk, sq], U8, isOutput=False)
        ot = nc.declare_dram_parameter("ot", [n_pairs, d, sq], F32, isOutput=True)
        with tile.TileContext(nc) as tc:
            emit_body(nc, tc, ot, qt, kt, v, mt, n_pairs, sq, sk, d, qn,
                      repeat=repeat, loop_n=loop_n, mmdt=mmdt)
    nc.compile()
    return nc


def _prep_inputs(query, key, value, dropout_mask, variant="v3"):
    """Host-side marshaling into per-core input maps."""
    import ml_dtypes

    q = np.asarray(query, dtype=np.float32).reshape(PAIRS, SQ, D)
    k = np.asarray(key, dtype=np.float32).reshape(PAIRS, SK, D)
    vv = np.asarray(value, dtype=np.float32).reshape(PAIRS, SK, D)
    m = np.asarray(dropout_mask).reshape(PAIRS, SQ, SK)

    qt = np.ascontiguousarray(q.transpose(0, 2, 1))  # [PAIRS, D, SQ]
    kt = np.ascontiguousarray(k.transpose(0, 2, 1))  # [PAIRS, D, SK]
    # V * SCALE rearranged: vr[p][r][c*D+j] = V[c*128+r, j] * SCALE
    vr = (vv * np.float32(SCALE)).reshape(PAIRS, SK // 128, 128, D)
    vr = np.ascontiguousarray(vr.transpose(0, 2, 1, 3)).reshape(PAIRS, 128, (SK // 128) * D)
    if variant == "v3":
        qt = qt.astype(ml_dtypes.bfloat16)
        kt = kt.astype(ml_dtypes.bfloat16)
        vr = vr.astype(ml_dtypes.bfloat16)
    # mask^T as fp8 bytes
    mb = (m != 0).astype(np.uint8) * np.uint8(FP8_ONE)  # [PAIRS, SQ, SK] u8
    mbt = np.ascontiguousarray(mb.transpose(0, 2, 1))  # [PAIRS, SK, SQ]

    in_maps = []
    for c in range(N_CORES):
        s = slice(c * PAIRS_PER_CORE, (c + 1) * PAIRS_PER_CORE)
        in_maps.append(
            {
                "qt": qt[s],
                "kt": kt[s],
                "v": vr[s],
                "mt": mbt[s],
            }
        )
    return in_maps


def kernel(query, key, value, dropout_mask):
    global LAST_RESULTS
    variant = os.environ.get("KERNEL_VARIANT", "v3")
    in_maps = _prep_inputs(query, key, value, dropout_mask, variant)
    nc = build_nc(variant=variant)
    res = run_bass_kernel_spmd(nc, in_maps, list(range(N_CORES)), trace=False)
    LAST_RESULTS = res
    outs = np.concatenate([r["ot"] for r in res.results], axis=0)  # [PAIRS, D, SQ]
    out = outs.astype(np.float32).transpose(0, 2, 1).reshape(B, H, SQ, D)
    return np.ascontiguousarray(out)


# revision 4
# speedup vs baseline: 2.2043x; 1.2668x over previous
"""Trainium2 Bass kernel for masked-dropout attention-score matmul.

Computes, for q/k/v [B,H,S,D] and an int32 0/1 keep-mask [B,H,S,S]:

    out = ((q @ k^T) * sqrt(D) * 2 * mask) @ v        (2 = 1/(1-p_drop))

Strategy (8 NeuronCores, SPMD, no collectives):
  - Shard the 32 (b,h) pairs 4-per-core.
  - Per pair, compute S^T = K @ Q^T on the PE (so the second matmul can
    consume it as its moving operand without any on-chip transpose),
    apply the mask during the PSUM->SBUF eviction, and accumulate
    O^T = V^T @ S'^T on the PE.
  - The scale (2*sqrt(D)) is folded into V on the host; mask values are
    shipped as fp8(0/1) bytes; Q^T/K^T/V are host-rearranged so all
    device DMAs are fully contiguous.

v3 (default): bf16 matmul operands (PE ~10% faster than f32r, FWL on),
  [128,1024] two-bank PSUM score groups, and the mask-apply work is
  split across three engines to get the whole masking stage under the
  PE roofline:
    path A: DVE multiplies PSUM f32 scores by the fp8 mask -> bf16 SBUF
    path B: ACT evicts PSUM->bf16, Pool converts mask fp8->bf16, DVE
            multiplies bf16 x bf16 in its 2x mode
    path C: like B but the multiply runs on Pool (gpsimd)
  Output is shipped bf16 and upcast on the host.
"""

import os
import sys

sys.path.insert(0, "/opt/trn_rl_repo")

import numpy as np

import concourse.bacc as bacc
import concourse.bass as bass
import concourse.mybir as mybir
import concourse.tile as tile
from concourse.bass_utils import run_bass_kernel_spmd

B, H, SQ, SK, D = 2, 16, 2048, 2048, 128
P_DROP = 0.5
SCALE = float(D) ** 0.5 / (1.0 - P_DROP)  # folded into V on the host
N_CORES = 8
PAIRS = B * H
PAIRS_PER_CORE = PAIRS // N_CORES

F32 = mybir.dt.float32
F32R = mybir.dt.float32r
FP8 = mybir.dt.float8e4
U8 = mybir.dt.uint8
BF16 = mybir.dt.bfloat16

FP8_ONE = 0x38  # float8_e4m3 encoding of 1.0

# module-level handle for test.py to inspect timing after a traced run
LAST_RESULTS = None


def emit_body(nc, tc, ot, qt, kt, v, mt, n_pairs, sq, sk, d=D, qn=512, repeat=1,
              loop_n=1, mmdt=F32R):
    """v1 per-core program (kept for A/B reference).

    APs (all on this core's DRAM):
      qt [n_pairs, d,  sq]  f32  : Q^T per pair
      kt [n_pairs, d,  sk]  f32  : K^T per pair
      v  [n_pairs, 128, (sk//128)*d] : V rearranged, scaled
      mt [n_pairs, sk, sq]  u8   : mask^T as fp8 bytes (0x00 / 0x38)
      ot [n_pairs, d,  sq]  f32  : O^T output
    """
    nkc = sk // 128
    nqc = sq // qn

    import contextlib

    with contextlib.ExitStack() as ctx:
        qt_pool = ctx.enter_context(tc.tile_pool(name="qt", bufs=2))
        kt_pool = ctx.enter_context(tc.tile_pool(name="kt", bufs=2))
        v_pool = ctx.enter_context(tc.tile_pool(name="v", bufs=2))
        m_pool = ctx.enter_context(tc.tile_pool(name="m", bufs=4))
        sp_pool = ctx.enter_context(tc.tile_pool(name="sp", bufs=6))
        o_pool = ctx.enter_context(tc.tile_pool(name="o", bufs=2))
        st_pool = ctx.enter_context(tc.tile_pool(name="st", bufs=4, space="PSUM"))
        ot_pool = ctx.enter_context(tc.tile_pool(name="otp", bufs=1, space="PSUM"))

        loop_cm = tc.For_i(0, loop_n, 1) if loop_n > 1 else contextlib.nullcontext()
        with loop_cm:
          for p in [pp for _ in range(repeat) for pp in range(n_pairs)]:
            qt_t = qt_pool.tile([128, sq], mmdt)
            nc.sync.dma_start(out=qt_t[:d], in_=qt[p])
            kt_t = kt_pool.tile([128, sk], mmdt)
            nc.sync.dma_start(out=kt_t[:d], in_=kt[p])
            v_t = v_pool.tile([128, nkc * d], mmdt)
            nc.sync.dma_start(out=v_t[:], in_=v[p])

            ot_ps = ot_pool.tile([128, sq], F32)

            for kc in range(nkc):
                m_t = m_pool.tile([128, sq], U8)
                nc.sync.dma_start(out=m_t[:], in_=mt[p, kc * 128 : (kc + 1) * 128, :])

                for qc in range(nqc):
                    st = st_pool.tile([128, qn], F32)
                    nc.tensor.matmul(
                        st[:],
                        kt_t[:d, kc * 128 : (kc + 1) * 128],
                        qt_t[:d, qc * qn : (qc + 1) * qn],
                        start=True,
                        stop=True,
                    )
                    sp = sp_pool.tile([128, qn], mmdt)
                    nc.vector.tensor_mul(
                        sp[:],
                        st[:],
                        m_t[:, qc * qn : (qc + 1) * qn].bitcast(FP8),
                    )
                    nc.tensor.matmul(
                        ot_ps[:d, qc * qn : (qc + 1) * qn],
                        v_t[:, kc * d : (kc + 1) * d],
                        sp[:],
                        start=(kc == 0),
                        stop=(kc == nkc - 1),
                    )

            o_t = o_pool.tile([128, sq], F32)
            nc.scalar.copy(o_t[:d], ot_ps[:d])
            nc.sync.dma_start(out=ot[p], in_=o_t[:d])


KC_PATHS = "ABBABABBABABBABB"  # 7 A / 9 B per 16 k-blocks


def emit_body_v3(nc, tc, ot, qt, kt, v, mt, n_pairs, sq, sk, d=D, qn=512,
                 loop_n=1, kc_paths=KC_PATHS, st_bufs=6):
    """Two-path masking split, bf16 matmuls, bf16 output, q-half passes.

    The q axis is processed in two half-passes so the O^T accumulator only
    occupies 2 PSUM banks, leaving 6 banks of [128,512] score tiles — a
    6-deep PE run-ahead that hides the eviction-chain latency.

    Per k-block (one [128, sq/2] mask half-slice, two [128,512] score
    groups), the PSUM->SBUF eviction + mask-multiply goes down one of:
      A: DVE tensor_mul(PSUM f32, fp8 mask) -> bf16 SBUF      (~650ns DVE)
      B: gpsimd cast-DMA ships the mask slice as bf16 (HBM bytes stay fp8),
         ACT evicts PSUM f32 -> bf16, DVE multiplies bf16 x bf16 in 2x mode
         (~550ns ACT + ~250ns DVE)
    The A:B ratio balances DVE against ACT so both stay under the PE time.
    """
    nkc = sk // 128
    hn = sq // 2
    ngc = hn // qn
    import contextlib

    with contextlib.ExitStack() as ctx:
        qt_pool = ctx.enter_context(tc.tile_pool(name="qt", bufs=2))
        kt_pool = ctx.enter_context(tc.tile_pool(name="kt", bufs=2))
        v_pool = ctx.enter_context(tc.tile_pool(name="v", bufs=2))
        m_pool = ctx.enter_context(tc.tile_pool(name="m", bufs=4))
        mb_pool = ctx.enter_context(tc.tile_pool(name="mb", bufs=4))
        se_pool = ctx.enter_context(tc.tile_pool(name="se", bufs=6))
        sp_pool = ctx.enter_context(tc.tile_pool(name="sp", bufs=8))
        o_pool = ctx.enter_context(tc.tile_pool(name="o", bufs=2))
        st_pool = ctx.enter_context(tc.tile_pool(name="st", bufs=st_bufs, space="PSUM"))
        ot_pool = ctx.enter_context(tc.tile_pool(name="otp", bufs=1, space="PSUM"))

        loop_cm = tc.For_i(0, loop_n, 1) if loop_n > 1 else contextlib.nullcontext()
        with loop_cm:
          for p in range(n_pairs):
            qt_t = qt_pool.tile([128, sq], BF16)
            nc.sync.dma_start(out=qt_t[:d], in_=qt[p])
            kt_t = kt_pool.tile([128, sk], BF16)
            nc.sync.dma_start(out=kt_t[:d], in_=kt[p])
            v_t = v_pool.tile([128, nkc * d], BF16)
            nc.sync.dma_start(out=v_t[:], in_=v[p])

            for h in range(2):
              q0 = h * hn
              ot_ps = ot_pool.tile([128, hn], F32)

              for kc in range(nkc):
                path = kc_paths[kc % len(kc_paths)]
                m_src = mt[p, kc * 128 : (kc + 1) * 128, q0 : q0 + hn]
                if path == "A":
                    m_t = m_pool.tile([128, hn], U8)
                    nc.sync.dma_start(out=m_t[:], in_=m_src)
                else:
                    mb_t = mb_pool.tile([128, hn], BF16)
                    nc.gpsimd.dma_start(out=mb_t[:], in_=m_src.bitcast(FP8))

                for g in range(ngc):
                    st = st_pool.tile([128, qn], F32)
                    nc.tensor.matmul(
                        st[:],
                        kt_t[:d, kc * 128 : (kc + 1) * 128],
                        qt_t[:d, q0 + g * qn : q0 + (g + 1) * qn],
                        start=True,
                        stop=True,
                    )
                    sp = sp_pool.tile([128, qn], BF16)
                    if path == "A":
                        nc.vector.tensor_mul(
                            sp[:], st[:], m_t[:, g * qn : (g + 1) * qn].bitcast(FP8)
                        )
                    else:
                        se = se_pool.tile([128, qn], BF16)
                        nc.scalar.copy(se[:], st[:])
                        nc.vector.tensor_mul(
                            sp[:], se[:], mb_t[:, g * qn : (g + 1) * qn]
                        )
                    nc.tensor.matmul(
                        ot_ps[:d, g * qn : (g + 1) * qn],
                        v_t[:, kc * d : (kc + 1) * d],
                        sp[:],
                        start=(kc == 0),
                        stop=(kc == nkc - 1),
                    )

              o_t = o_pool.tile([128, hn], BF16)
              nc.scalar.copy(o_t[:d], ot_ps[:d])
              nc.sync.dma_start(out=ot[p, :, q0 : q0 + hn], in_=o_t[:d])


def build_nc(n_pairs=PAIRS_PER_CORE, sq=SQ, sk=SK, d=D, qn=512, variant="v3",
             repeat=1, loop_n=1):
    nc = bacc.Bacc("TRN2", target_bir_lowering=False, debug=False)
    if variant == "v3":
        qt = nc.declare_dram_parameter("qt", [n_pairs, d, sq], BF16, isOutput=False)
        kt = nc.declare_dram_parameter("kt", [n_pairs, d, sk], BF16, isOutput=False)
        v = nc.declare_dram_parameter("v", [n_pairs, 128, (sk // 128) * d], BF16,
                                      isOutput=False)
        mt = nc.declare_dram_parameter("mt", [n_pairs, sk, sq], U8, isOutput=False)
        ot = nc.declare_dram_parameter("ot", [n_pairs, d, sq], BF16, isOutput=True)
        with tile.TileContext(nc) as tc:
            emit_body_v3(nc, tc, ot, qt, kt, v, mt, n_pairs, sq, sk, d,
                         loop_n=loop_n)
    else:
        mmdt = F32R
        qt = nc.declare_dram_parameter("qt", [n_pairs, d, sq], mmdt, isOutput=False)
        kt = nc.declare_dram_parameter("kt", [n_pairs, d, sk], mmdt, isOutput=False)
        v = nc.declare_dram_parameter("v", [n_pairs, 128, (sk // 128) * d], mmdt,
                                      isOutput=False)
        mt = nc.declare_dram_parameter("mt", [n_pairs, sk, sq], U8, isOutput=False)
        ot = nc.declare_dram_parameter("ot", [n_pairs, d, sq], F32, isOutput=True)
        with tile.TileContext(nc) as tc:
            emit_body(nc, tc, ot, qt, kt, v, mt, n_pairs, sq, sk, d, qn,
                      repeat=repeat, loop_n=loop_n, mmdt=mmdt)
    nc.compile()
    return nc


def _prep_inputs(query, key, value, dropout_mask, variant="v3"):
    """Host-side marshaling into per-core input maps."""
    import ml_dtypes

    q = np.asarray(query, dtype=np.float32).reshape(PAIRS, SQ, D)
    k = np.asarray(key, dtype=np.float32).reshape(PAIRS, SK, D)
    vv = np.asarray(value, dtype=np.float32).reshape(PAIRS, SK, D)
    m = np.asarray(dropout_mask).reshape(PAIRS, SQ, SK)

    qt = np.ascontiguousarray(q.transpose(0, 2, 1))  # [PAIRS, D, SQ]
    kt = np.ascontiguousarray(k.transpose(0, 2, 1))  # [PAIRS, D, SK]
    # V * SCALE rearranged: vr[p][r][c*D+j] = V[c*128+r, j] * SCALE
    vr = (vv * np.float32(SCALE)).reshape(PAIRS, SK // 128, 128, D)
    vr = np.ascontiguousarray(vr.transpose(0, 2, 1, 3)).reshape(PAIRS, 128, (SK // 128) * D)
    if variant == "v3":
        qt = qt.astype(ml_dtypes.bfloat16)
        kt = kt.astype(ml_dtypes.bfloat16)
        vr = vr.astype(ml_dtypes.bfloat16)
    # mask^T as fp8 bytes
    mb = (m != 0).astype(np.uint8) * np.uint8(FP8_ONE)  # [PAIRS, SQ, SK] u8
    mbt = np.ascontiguousarray(mb.transpose(0, 2, 1))  # [PAIRS, SK, SQ]

    in_maps = []
    for c in range(N_CORES):
        s = slice(c * PAIRS_PER_CORE, (c + 1) * PAIRS_PER_CORE)
        in_maps.append(
            {
                "qt": qt[s],
                "kt": kt[s],
                "v": vr[s],
                "mt": mbt[s],
            }
        )
    return in_maps


def kernel(query, key, value, dropout_mask):
    global LAST_RESULTS
    variant = os.environ.get("KERNEL_VARIANT", "v3")
    in_maps = _prep_inputs(query, key, value, dropout_mask, variant)
    nc = build_nc(variant=variant)
    res = run_bass_kernel_spmd(nc, in_maps, list(range(N_CORES)), trace=False)
    LAST_RESULTS = res
    outs = np.concatenate([r["ot"] for r in res.results], axis=0)  # [PAIRS, D, SQ]
    out = outs.astype(np.float32).transpose(0, 2, 1).reshape(B, H, SQ, D)
    return np.ascontiguousarray(out)
